# revision 21
# baseline (speedup 1.0000x reference)
"""AR block LSTM on 8 TRN2 NeuronCores.

Data-parallel over batch (1024 -> 128/core), weights replicated.
Per core, each LSTM step computes z = [x;1] @ [Wx;b] + h @ Uh into 8 PSUM
banks (4096 gate cols, native i|f|g|o order), does the cell math on
ACT/DVE in unit-halves, and transposes h2 back to [units, batch] layout
with PE-transposes so it can serve as the next step's stationary operand.
The AR phase computes pT = (h @ Wd + bd)^T with 8 small matmuls; pT is both
the output block and the next step's x input (so no feedback transpose).

Matmuls run in bf16 (near the 213 ns/MM N=512 PE roofline; end-to-end
rel err ~4e-3 vs the fp32 reference); the cell state c stays fp32.
During warmup, half-B transposes are deferred into the next step's
matmul stream so the in-order PE does not stall on the ACT/DVE cell
math. Measured on silicon: 2.525 ms whole-NEFF exec across 8 cores.
"""
import os
import numpy as np
import ml_dtypes

import concourse.bass as bass
import concourse.mybir as mybir
import concourse.tile as tile
from concourse.tile_rust import add_dep_helper
from concourse import bacc
from concourse.bass_utils import run_bass_kernel_spmd

F32 = mybir.dt.float32
BF16 = mybir.dt.bfloat16

N_CORES = 8
BATCH = 1024
B = BATCH // N_CORES          # 128 batch rows per core
WARM_T = 128
UNITS = 1024
KT = UNITS // 128             # 8 K-tiles
G = 4 * UNITS                 # 4096 gate cols
NBANK = G // 512              # 8 PSUM banks of 512 gate cols
FEAT = 8
NBLK = 192 // FEAT            # 24 output blocks
CH = 16                       # xT chunk length (warmup steps per DMA)

ACT = mybir.ActivationFunctionType


def build(nwarm=WARM_T, nar=NBLK - 1):
    nc = bacc.Bacc("TRN2", target_bir_lowering=False, debug=False,
                   num_devices=N_CORES)
    xt_e = nc.dram_tensor("xt", [nwarm, FEAT + 1, B], BF16, kind="ExternalInput").ap()
    uh_e = nc.dram_tensor("uh", [KT, 128, G], BF16, kind="ExternalInput").ap()
    wxa_e = nc.dram_tensor("wxa", [FEAT + 1, G], BF16, kind="ExternalInput").ap()
    wd_e = nc.dram_tensor("wd", [KT, 128, FEAT], BF16, kind="ExternalInput").ap()
    bd_e = nc.dram_tensor("bdv", [FEAT, 1], F32, kind="ExternalInput").ap()
    id_e = nc.dram_tensor("ident", [128, 128], BF16, kind="ExternalInput").ap()
    out_e = nc.dram_tensor("out", [nar + 1, FEAT, B], F32, kind="ExternalOutput").ap()

    nsteps = nwarm + nar

    with tile.TileContext(nc) as tc:
        with tc.tile_pool(name="w", bufs=1) as wp, \
             tc.tile_pool(name="xt", bufs=2) as xtp, \
             tc.tile_pool(name="ht", bufs=2) as htp, \
             tc.tile_pool(name="st", bufs=1) as stp, \
             tc.tile_pool(name="h2", bufs=2) as h2p, \
             tc.tile_pool(name="gate", bufs=6) as gp, \
             tc.tile_pool(name="m", bufs=4) as mp, \
             tc.tile_pool(name="pt", bufs=2) as ptp, \
             tc.tile_pool(name="z", bufs=6, space="PSUM") as zp, \
             tc.tile_pool(name="tr", bufs=2, space="PSUM") as trp:

            # ---- resident weights -------------------------------------
            uh_sb = []
            for k in range(KT):
                u1 = wp.tile([128, G], BF16, tag=f"uh{k}")
                nc.sync.dma_start(out=u1[:], in_=uh_e[k])
                uh_sb.append(u1)
            # Wx+bias rows replicated at partitions 0/32/64/96 for row-packed
            # K=9 matmuls via tile_position.
            wxa_sb = wp.tile([128, G], BF16)
            for r in range(4):
                nc.sync.dma_start(out=wxa_sb[32 * r:32 * r + FEAT + 1, :], in_=wxa_e[:])
            wd_sb = wp.tile([128, KT, FEAT], BF16)
            for k in range(KT):
                nc.sync.dma_start(out=wd_sb[:, k, :], in_=wd_e[k])
            bd_sb = wp.tile([FEAT, 1], F32)
            nc.sync.dma_start(out=bd_sb[:], in_=bd_e[:])
            id_sb = wp.tile([128, 128], BF16)
            nc.sync.dma_start(out=id_sb[:], in_=id_e[:])

            # ---- state ------------------------------------------------
            hT = htp.tile([128, KT, B], BF16, tag="hT")
            nc.gpsimd.memset(hT[:], 0.0)
            c_sb = stp.tile([128, UNITS], F32)
            nc.gpsimd.memset(c_sb[:], 0.0)

            # ---- xT chunk prefetch ------------------------------------
            nchunk = (nwarm + CH - 1) // CH
            chunk_tiles = {}

            def load_chunk(ci):
                t0 = ci * CH
                n = min(CH, nwarm - t0)
                tl = xtp.tile([128, CH, B], BF16, tag="xt")
                for r in range(4):
                    nc.sync.dma_start(
                        out=tl[32 * r:32 * r + FEAT + 1, :n, :],
                        in_=xt_e[t0:t0 + n].rearrange("t p b -> p t b"))
                return tl

            if nwarm > 0:
                chunk_tiles[0] = load_chunk(0)

            def p_block(j, hT_new):
                """pT = (h_new @ Wd + bd)^T -> out block j; returns pT_aug."""
                pp = trp.tile([FEAT, B], F32, tag="tr")
                for k in range(KT):
                    nc.tensor.matmul(pp[:], wd_sb[:, k, :], hT_new[:, k, :],
                                     start=(k == 0), stop=(k == KT - 1))
                pto = ptp.tile([FEAT, B], F32, tag="pto")
                nc.scalar.activation(pto[:], pp[:], ACT.Identity, bias=bd_sb[:])
                nc.sync.dma_start(out=out_e[j], in_=pto[:])
                pta = ptp.tile([128, B], BF16, tag="pta")
                nc.gpsimd.memset(pta[:], 1.0)
                for r in range(4):
                    nc.vector.tensor_copy(pta[32 * r:32 * r + FEAT, :], pto[:])
                return pta

            pta = None
            pending_tr = None
            bank_order = [0, 2, 4, 6, 1, 3, 5, 7]

            for t in range(nsteps):
                warm = t < nwarm
                if warm:
                    ci, s = divmod(t, CH)
                    if s == 0 and ci + 1 < nchunk:
                        chunk_tiles[ci + 1] = load_chunk(ci + 1)
                        chunk_tiles.pop(ci - 1, None)
                    ct = chunk_tiles[ci]
                    xaug = lambda r: ct[32 * r:32 * r + FEAT + 1, s, :]
                else:
                    cpta = pta
                    xaug = lambda r: cpta[32 * r:32 * r + FEAT + 1, :]

                # ---- z matmuls, bank-major so early banks finish early
                zt = {}
                h2 = h2p.tile([128, UNITS], BF16, tag="h2")
                hT_new = htp.tile([128, KT, B], BF16, tag="hT")

                def half_chain(h):
                    u0 = h * 512
                    si = gp.tile([128, 512], F32, tag="g")
                    sf = gp.tile([128, 512], F32, tag="g")
                    tg = gp.tile([128, 512], F32, tag="g")
                    so = gp.tile([128, 512], F32, tag="g")
                    nc.scalar.activation(si[:], zt[0 + h][:], ACT.Sigmoid)
                    nc.scalar.activation(sf[:], zt[2 + h][:], ACT.Sigmoid)
                    nc.scalar.activation(tg[:], zt[4 + h][:], ACT.Tanh)
                    nc.scalar.activation(so[:], zt[6 + h][:], ACT.Sigmoid)
                    m1 = mp.tile([128, 512], F32, tag="m")
                    m2 = mp.tile([128, 512], F32, tag="m")
                    nc.vector.tensor_mul(m1[:], sf[:], c_sb[:, u0:u0 + 512])
                    nc.vector.tensor_mul(m2[:], si[:], tg[:])
                    nc.vector.tensor_add(c_sb[:, u0:u0 + 512], m1[:], m2[:])
                    tc2 = gp.tile([128, 512], F32, tag="g")
                    nc.scalar.activation(tc2[:], c_sb[:, u0:u0 + 512], ACT.Tanh)
                    nc.vector.tensor_mul(h2[:, u0:u0 + 512], so[:], tc2[:])

                def emit_tr(h, h2t, hTn):
                    insts = []
                    for k in range(4 * h, 4 * h + 4):
                        tr = trp.tile([128, 128], BF16, tag="tr")
                        ti = nc.tensor.transpose(tr[:], h2t[:, k * 128:(k + 1) * 128],
                                                 id_sb[:])
                        insts.append(ti)
                        nc.vector.tensor_copy(hTn[:, k, :], tr[:])
                    return insts

                def kmms(bk, ks, first_start):
                    n0 = bk * 512
                    mi = None
                    for k in ks:
                        mi = nc.tensor.matmul(zt[bk][:], hT[:, k, :],
                                              uh_sb[k][:, n0:n0 + 512],
                                              start=(first_start and k == ks[0]),
                                              stop=(warm and k == ks[-1]))
                    return mi

                def packs(banks, first):
                    # Row-packed x/bias matmuls: K=9 tiles at PE rows
                    # 0/32/64/96 run concurrently, each into its own PSUM
                    # bank. During warmup x is ready early, so they open the
                    # accumulation groups; in the AR phase x (= pT) arrives
                    # late, so they close them instead.
                    for r, bk in enumerate(banks):
                        n0r = bk * 512
                        nc.tensor.matmul(
                            zt[bk][:],
                            xaug(r),
                            wxa_sb[32 * r:32 * r + FEAT + 1, n0r:n0r + 512],
                            start=first, stop=(t == 0 or not first),
                            tile_position=(32 * r, 0))

                for bk in bank_order:
                    ztile = zp.tile([128, 512], F32, tag="z")
                    zt[bk] = ztile
                bA, bB = bank_order[:4], bank_order[4:]

                if pending_tr is not None:
                    # Finish the previous step's half-B transposes in the
                    # middle of this step's half-A matmuls: k-tiles 0..3 for
                    # all four banks (~3.4us of PE work) only need half-A
                    # k-tiles, which covers the cell-math latency of the
                    # previous half-B chain so the PE never stalls.
                    assert warm and t > 0
                    packs(bA, first=True)
                    cov = None
                    for bk in bA:
                        cov = kmms(bk, [0, 1, 2, 3], False)
                    trs = pending_tr()
                    pending_tr = None
                    # The scheduler otherwise hoists these transposes ahead of
                    # the covering matmuls and the in-order PE then stalls on
                    # the cell math.
                    for ti in trs:
                        add_dep_helper(ti.ins, cov.ins, sync=False,
                                       reason="cover deferred trB behind kA k0-3")
                    for bk in bA:
                        kmms(bk, [4, 5, 6, 7], False)
                elif warm:
                    packs(bA, first=True)
                    if t > 0:
                        for bk in bA:
                            kmms(bk, list(range(KT)), False)
                else:
                    for bk in bA:
                        kmms(bk, list(range(KT)), True)
                    packs(bA, first=False)
                half_chain(0)

                if warm:
                    packs(bB, first=True)
                    if t > 0:
                        for bk in bB:
                            kmms(bk, list(range(KT)), False)
                else:
                    for bk in bB:
                        kmms(bk, list(range(KT)), True)
                    packs(bB, first=False)
                emit_tr(0, h2, hT_new)
                half_chain(1)

                if warm and t < nwarm - 1:
                    # defer half-B transposes into the next step's stream
                    ch2, chT = h2, hT_new
                    pending_tr = lambda: emit_tr(1, ch2, chT)
                else:
                    emit_tr(1, h2, hT_new)

                hT = hT_new
                if t >= nwarm - 1:
                    pta = p_block(t - nwarm + 1, hT_new)

    nc.finalize()
    return nc


_NC_CACHE = {}


def _get_nc(nwarm, nar):
    key = (nwarm, nar)
    if key not in _NC_CACHE:
        _NC_CACHE[key] = build(nwarm, nar)
    return _NC_CACHE[key]


def prep_inputs(inputs, Wx, Uh, b, Wd, bd, nwarm=WARM_T):
    """Host-side prep: shard + transpose + bf16. Returns in_maps list."""
    bf = ml_dtypes.bfloat16
    uh = np.ascontiguousarray(
        Uh.astype(np.float32).reshape(KT, 128, G)).astype(bf)
    wxa = np.concatenate(
        [Wx.astype(np.float32), b.astype(np.float32)[None, :]], axis=0).astype(bf)
    wd = np.ascontiguousarray(
        Wd.astype(np.float32).reshape(KT, 128, FEAT)).astype(bf)
    bdv = np.ascontiguousarray(bd.astype(np.float32).reshape(FEAT, 1))
    ident = np.eye(128, dtype=np.float32).astype(bf)

    in_maps = []
    for ci in range(N_CORES):
        shard = np.asarray(inputs[ci * B:(ci + 1) * B, :nwarm, :], dtype=np.float32)
        # [B, T, F] -> [T, F, B], then append the ones row -> [T, F+1, B]
        xt = np.transpose(shard, (1, 2, 0))
        xt = np.concatenate([xt, np.ones((nwarm, 1, B), np.float32)], axis=1)
        in_maps.append({
            "xt": np.ascontiguousarray(xt).astype(bf),
            "uh": uh, "wxa": wxa, "wd": wd, "bdv": bdv, "ident": ident,
        })
    return in_maps


def run(inputs, Wx, Uh, b, Wd, bd, nwarm=WARM_T, nar=NBLK - 1, trace=False):
    nc = _get_nc(nwarm, nar)
    in_maps = prep_inputs(inputs, Wx, Uh, b, Wd, bd, nwarm)
    res = run_bass_kernel_spmd(nc, in_maps, core_ids=list(range(N_CORES)),
                               trace=trace)
    outs = []
    for ci in range(N_CORES):
        o = res.results[ci]["out"]          # [nblk, FEAT, B]
        outs.append(np.transpose(o, (2, 0, 1)).reshape(B, (nar + 1) * FEAT, 1))
    full = np.concatenate(outs, axis=0).astype(np.float32)
    return full, res


def kernel(inputs, Wx, Uh, b, Wd, bd):
    full, _ = run(np.asarray(inputs), np.asarray(Wx), np.asarray(Uh),
                  np.asarray(b), np.asarray(Wd), np.asarray(bd))
    return full


if __name__ == "__main__":
    rng = np.random.default_rng(0)
    s = 0.05
    inputs = rng.standard_normal((BATCH, WARM_T, FEAT)).astype(np.float32)
    Wx = (rng.standard_normal((FEAT, G)) * s).astype(np.float32)
    Uh = (rng.standard_normal((UNITS, G)) * s).astype(np.float32)
    b = np.zeros(G, np.float32)
    Wd = (rng.standard_normal((UNITS, FEAT)) * s).astype(np.float32)
    bd = np.zeros(FEAT, np.float32)
    out = kernel(inputs=inputs, Wx=Wx, Uh=Uh, b=b, Wd=Wd, bd=bd)
    print("out shape:", out.shape, out.dtype)
